# revision 23
# baseline (speedup 1.0000x reference)
"""Trainium2 Bass kernel: BiologicalAttention (mask-modulated multi-head attention).

Full computation:
    qkv = x @ W_qkv + b_qkv                         [B, N, 3, H, D]
    S   = (q @ k^T) * D**-0.5 * (0.1 + 0.9*mask)    [B, H, N, N]
    P   = softmax(S, axis=-1)
    out = (P @ v) reshaped to [B, N, C]
    y   = out @ W_out + b_out

Sharding (8 cores): core c handles batch b = c//2 and a 4-head group
g = c%2 (heads 4g..4g+3).  Each core computes a partial y for its batch
(its heads' contribution to the output projection); the host sums the
two partials per batch and adds b_out.

On-core layout / engine assignment (v2):
  - All attention operands are bf16 (x, q, k, v, mask, P); PSUM
    accumulation stays fp32, output y fp32.  The DVE's fp32 tensor_tensor
    from PSUM runs at 1x (plus a pipe-drain tax), so element work is
    spread: per key-tile the pre-softmax mask multiply runs either on the
    DVE (tensor_mul straight out of PSUM) or on GPSIMD (ACT evicts
    PSUM->SBUF bf16, Q7 multiplies in place).  gpx/16 of tiles go to GPSIMD.
  - Scores are computed TRANSPOSED: T[m, n] = sum_d k[m,d] q[n,d], so the
    softmax denominator (sum over keys m = partitions) comes from a matmul:
    V is stored [m, d] with a ones-column appended, so P@[v|1] yields both
    the attention output (rows 0..31) and the softmax sums (row 32).
  - exp runs on the scalar engine, one op per TE-tile group.
  - P@V for group g is drained three groups late so the PE never waits on
    the DVE/ACT softmax chain; a pass's tail groups (and the whole last
    pass of a rep) drain under the next pass's/rep's leading QK^T work.
    Two heads share each 1-bank PSUM accumulator (tile_position col 0/64),
    so the accumulators double-buffer across passes in 4 banks total.
  - Normalization is deferred past P@V: 1/sums via DVE reciprocal (keeps
    the ACT on a single Exp table set), O_h scaled while evicting PSUM.
  - Host pre-folds scale into the mask: maskT = ((0.1+0.9*mask)*D^-0.5).T
    in bf16, and pre-augments weights with bias rows (x^T gets a ones row).
"""

import numpy as np
from contextlib import ExitStack

import concourse.bass as bass
import concourse.tile as tile
import concourse.mybir as mybir
from concourse import bacc

f32 = mybir.dt.float32
f32r = mybir.dt.float32r
bf16 = mybir.dt.bfloat16
Act = mybir.ActivationFunctionType

# problem shape (hardcoded per contract)
B, N, C, H = 4, 2048, 256, 8
D = 32
SCALE = D ** -0.5
HPC = 4                # heads per core
HD = HPC * D           # 128
VW = HPC * (D + 1)     # 132: per-m-tile v-store width ([v_h | 1] x 4 heads)
NCORES = 8


def build_program(n=N, debug=False, reps=1, gpx=7, act_off=0):
    """Build the SPMD Bass program for one core's shard. Same program runs
    on all 8 cores with different input bindings.

    reps: repeat the whole computation (timing aid).
    gpx: of every 16 key-tiles, gpx are mask-multiplied on GPSIMD (ACT
    evicts PSUM->SBUF, Q7 multiplies in SBUF); the rest on the DVE.
    act_off: move the small epilogue copies (softmax sums, y eviction)
    from the ACT to the DVE.
    """
    NQ = 4                 # n (query) passes
    CH = n // NQ           # 512 at full size
    MT = n // 128          # m-tiles (key tiles)
    TE = 2                 # m-tiles sharing one Exp op
    assert MT % TE == 0

    nc = bacc.Bacc("TRN2", target_bir_lowering=False, debug=debug)

    xT_d = nc.dram_tensor("xT", [C, n], bf16, kind="ExternalInput")
    maskT_d = nc.dram_tensor("maskT", [n, n], bf16, kind="ExternalInput")
    wq_d = nc.dram_tensor("wq", [C + 1, HD], bf16, kind="ExternalInput")
    wk_d = nc.dram_tensor("wk", [C + 1, HD], bf16, kind="ExternalInput")
    wv_d = nc.dram_tensor("wv", [C + 1, VW], bf16, kind="ExternalInput")
    wo_d = nc.dram_tensor("wo", [HD, C], f32, kind="ExternalInput")
    y_d = nc.dram_tensor("y", [n, C], f32, kind="ExternalOutput")

    def is_gp(t):
        return (t * gpx) % MT < gpx

    with tile.TileContext(nc) as tc, ExitStack() as ctx:
        const = ctx.enter_context(tc.tile_pool(name="const", bufs=1))
        maskp = ctx.enter_context(tc.tile_pool(name="maskp", bufs=8))
        tpool = ctx.enter_context(tc.tile_pool(name="tpool", bufs=4))
        ppool = ctx.enter_context(tc.tile_pool(name="ppool", bufs=6))
        ypool = ctx.enter_context(tc.tile_pool(name="ypool", bufs=2))
        spool = ctx.enter_context(tc.tile_pool(name="spool", bufs=2))
        rpool = ctx.enter_context(tc.tile_pool(name="rpool", bufs=2))
        psT = ctx.enter_context(tc.tile_pool(name="psT", bufs=2, space="PSUM"))
        # two 2-bank pools; po(q) lives in pool q%2, and pass q's projection
        # output py(q) reuses the same slots right after the normalize reads,
        # so consecutive passes' PV accumulators never share banks.
        psA = ctx.enter_context(tc.tile_pool(name="psA", bufs=2, space="PSUM"))
        psB = ctx.enter_context(tc.tile_pool(name="psB", bufs=2, space="PSUM"))
        psO2 = (psA, psB)

        # ---------------- constants / inputs ----------------
        xb0 = const.tile([128, n], bf16, tag="xb0")
        xb1 = const.tile([128, n], bf16, tag="xb1")
        nc.sync.dma_start(xb0[:], xT_d[0:128, :])
        nc.sync.dma_start(xb1[:], xT_d[128:256, :])
        ones_b = const.tile([1, n], bf16, tag="ones_b")
        nc.vector.memset(ones_b[:], 1.0)
        zc = const.tile([1, 128], bf16, tag="zc")
        nc.vector.memset(zc[:], 0.0)
        zrow = const.tile([1, CH], bf16, tag="zrow")
        nc.vector.memset(zrow[:], 0.0)

        wq_sb = const.tile([128, 2 * HD], bf16, tag="wq_sb")
        wk_sb = const.tile([128, 2 * HD], bf16, tag="wk_sb")
        wv_sb = const.tile([128, 2 * VW], bf16, tag="wv_sb")
        wqb = const.tile([1, HD], bf16, tag="wqb")
        wkb = const.tile([1, HD], bf16, tag="wkb")
        wvb = const.tile([1, VW], bf16, tag="wvb")
        wo_f = const.tile([128, C], f32, tag="wo_f")
        wo_sb = const.tile([128, C], f32r, tag="wo_sb")
        for sb, d_, w in ((wq_sb, wq_d, HD), (wk_sb, wk_d, HD), (wv_sb, wv_d, VW)):
            nc.sync.dma_start(sb[:, 0:w], d_[0:128, :])
            nc.sync.dma_start(sb[:, w:2 * w], d_[128:256, :])
        nc.sync.dma_start(wqb[:], wq_d[256:257, :])
        nc.sync.dma_start(wkb[:], wk_d[256:257, :])
        nc.sync.dma_start(wvb[:], wv_d[256:257, :])
        nc.sync.dma_start(wo_f[:], wo_d[:])
        nc.scalar.copy(wo_sb[:], wo_f[:])

        qT_sb = const.tile([128, n], bf16, tag="qT_sb")
        kT_sb = const.tile([128, n], bf16, tag="kT_sb")
        # v_store is double-buffered by rep parity: the tail P@V of rep i
        # drains under rep i+1's phase 1, which writes the other buffer.
        v_stores = [const.tile([128, MT * VW], bf16, name=f"v_store{i}",
                               tag=f"v_store{i}") for i in range(2)]
        O_allT = const.tile([128, n], f32r, tag="O_allT")

        pending = None
        po = None
        for _rep in range(reps):
            v_store = v_stores[_rep % 2]
            # ------------- phase 1: QKV projections (bf16) -------------
            # qT/kT: [32h+d, n] = W.T @ x.T (+ bias via ones-row rank-1 term)
            for chunk in range(NQ):
                cs = bass.ts(chunk, CH)
                for dst, w_sb, w_b in ((qT_sb, wq_sb, wqb), (kT_sb, wk_sb, wkb)):
                    pq = psT.tile([128, CH], f32, name="pq", tag="psT")
                    nc.tensor.matmul(pq[:], lhsT=w_sb[:, 0:HD], rhs=xb0[:, cs],
                                     start=True, stop=False)
                    nc.tensor.matmul(pq[:], lhsT=w_sb[:, HD:2 * HD],
                                     rhs=xb1[:, cs], start=False, stop=False)
                    nc.tensor.matmul(pq[:], lhsT=w_b[0:1, :],
                                     rhs=ones_b[0:1, cs],
                                     start=False, stop=True)
                    nc.vector.tensor_copy(dst[:, cs], pq[:])
            # v: [m, (v_h | 1) x 4] per m-tile; ones col comes from the bias row
            for t in range(MT):
                ms = bass.ts(t, 128)
                pv = psT.tile([128, VW], f32, name="pv", tag="psT")
                nc.tensor.matmul(pv[:], lhsT=xb0[:, ms], rhs=wv_sb[:, 0:VW],
                                 start=True, stop=False)
                nc.tensor.matmul(pv[:], lhsT=xb1[:, ms],
                                 rhs=wv_sb[:, VW:2 * VW],
                                 start=False, stop=False)
                nc.tensor.matmul(pv[:], lhsT=ones_b[0:1, ms], rhs=wvb[0:1, :],
                                 start=False, stop=True)
                nc.vector.tensor_copy(v_store[:, t * VW:(t + 1) * VW],
                                      pv[:])

            # ------------- phase 2: attention, one pass per n-chunk ----------
            # po is 2 PSUM banks (double-buffered across passes): heads h
            # pack two-per-bank at partition offsets 0/64 via tile_position,
            # so the normalize epilogue of pass q overlaps pass q+1 fully.
            def epilogue_a(q, po):
                qs = bass.ts(q, CH)
                # sums (psum row 32+64j of each head) -> partition 32h
                sraw = spool.tile([128, CH], f32, name="sraw", tag="sraw")
                sums_copy = (nc.vector.tensor_copy if act_off
                             else nc.scalar.copy)
                for h in range(4):
                    sums_copy(sraw[32 * h:32 * h + 1, :],
                              po[h // 2][64 * (h % 2) + 32:
                                         64 * (h % 2) + 33, :])
                # broadcast each head's sums row across its 32 partitions (DMA)
                r_raw = rpool.tile([128, CH], f32, name="r_raw", tag="r_raw")
                src = sraw[:]
                bc = bass.AP(tensor=src.tensor, offset=src.offset,
                             ap=[[32 * src.ap[0][0], 4], [0, 32], src.ap[-1]])
                nc.sync.dma_start(r_raw[:], bc)
                r_all = rpool.tile([128, CH], f32, name="r_all", tag="r_all")
                nc.vector.reciprocal(r_all[:], r_raw[:])
                # evict + normalize O^T chunks (rounds to f32r for projection)
                for h in range(4):
                    nc.vector.tensor_mul(O_allT[32 * h:32 * h + 32, qs],
                                         po[h // 2][64 * (h % 2):
                                                    64 * (h % 2) + 32, :],
                                         r_all[32 * h:32 * h + 32, :])

            def alloc_py(q):
                # projection PSUM for pass q: the same pool slots po(q) used
                # (free right after epilogue_a's normalize reads)
                pool = psO2[q % 2]
                return [pool.tile([128, CH], f32, name="py",
                                  tag=f"ps{'AB'[q % 2]}") for _ in range(2)]

            def epilogue_b(q, py):
                # output projection for this n-chunk
                for j in range(CH // 128):
                    ncol = q * CH + j * 128
                    nc.tensor.matmul(py[j // 2][:, (j % 2) * C:
                                                (j % 2 + 1) * C],
                                     lhsT=O_allT[:, ncol:ncol + 128],
                                     rhs=wo_sb[:], start=True, stop=True)
                y_sb = ypool.tile([128, (CH // 128) * C], f32, name="y_sb",
                                  tag="y_sb")
                y_copy = (nc.vector.tensor_copy if act_off
                          else nc.scalar.copy)
                y_copy(y_sb[:, 0:2 * C], py[0][:])
                y_copy(y_sb[:, 2 * C:4 * C], py[1][:])
                for j in range(CH // 128):
                    nc.sync.dma_start(
                        y_d[q * CH + j * 128:q * CH + (j + 1) * 128, :],
                        y_sb[:, j * C:(j + 1) * C])

            def drain_pv(po, g, P_t, vst=None):
                # P@V for TE-group g (tiles TE*g .. TE*g+TE-1).  Two heads
                # share a PSUM bank at partition offsets 0/64; the bank was
                # zero-initialized at alloc so every matmul accumulates.
                if vst is None:
                    vst = v_store
                for tj in range(TE):
                    tb = TE * g + tj
                    for h in range(4):
                        vs = vst[:, tb * VW + 33 * h:
                                 tb * VW + 33 * h + 33]
                        off = 64 * (h % 2)
                        nc.tensor.matmul(
                            po[h // 2][off:off + 33, :],
                            lhsT=vs,
                            rhs=P_t[:, (tj * 4 + h) * CH:
                                    (tj * 4 + h + 1) * CH],
                            start=False,
                            stop=(tb == MT - 1),
                            tile_position=(0, off),
                            skip_group_check=True)

            for q in range(NQ):
                backlog = []
                for t in range(MT):
                    mask_t = maskp.tile([128, CH], bf16, tag="mask_t")
                    nc.sync.dma_start(
                        mask_t[:],
                        maskT_d[t * 128:(t + 1) * 128, q * CH:(q + 1) * CH])
                    ti = t % TE
                    if ti == 0:
                        T_tile = tpool.tile([128, TE * 4 * CH], bf16,
                                            name="T_tile", tag="T_tile")
                        P_tile = ppool.tile([128, TE * 4 * CH], bf16,
                                            name="P_tile", tag="P_tile")
                    for pair in range(2):
                        pt = psT.tile([128, 2 * CH], f32, name="pt", tag="psT")
                        for hh in range(2):
                            h = 2 * pair + hh
                            nc.tensor.matmul(
                                pt[:, hh * CH:(hh + 1) * CH],
                                lhsT=kT_sb[32 * h:32 * h + 32,
                                           t * 128:(t + 1) * 128],
                                rhs=qT_sb[32 * h:32 * h + 32,
                                          bass.ts(q, CH)],
                                start=True, stop=True,
                                tile_position=(32 * h, 0))
                        # fused PSUM-evict + mask multiply (mask repeated 2x)
                        mrep = bass.AP(tensor=mask_t[:].tensor,
                                       offset=mask_t[:].offset,
                                       ap=[mask_t[:].ap[0], [0, 2],
                                           mask_t[:].ap[-1]])
                        dst = T_tile[:, (ti * 4 + pair * 2) * CH:
                                     (ti * 4 + pair * 2 + 2) * CH]
                        if is_gp(t):
                            # relieve the DVE: ACT evicts PSUM, GPSIMD
                            # does the elementwise multiply in SBUF
                            nc.scalar.copy(dst, pt[:])
                            nc.gpsimd.tensor_mul(dst, dst, mrep)
                        else:
                            nc.vector.tensor_mul(dst, pt[:], mrep)
                    if ti == TE - 1:
                        g = t // TE
                        nc.scalar.activation(P_tile[:], T_tile[:], Act.Exp)
                        backlog.append((g, P_tile))
                        if g == 0:
                            # previous pass's P@V tail (three groups) and
                            # its epilogue run here, hidden behind this
                            # pass's first QK^T groups
                            py_prev = None
                            po = [psO2[q % 2].tile(
                                [128, CH], f32, name="po",
                                tag=f"ps{'AB'[q % 2]}") for _ in range(2)]
                            for pb in po:
                                nc.tensor.matmul(pb[:], lhsT=zc[0:1, :],
                                                 rhs=zrow[0:1, :],
                                                 start=True, stop=True)
                        if g <= 2 and pending is not None:
                            q_prev, po_prev, bl_prev, vs_prev = pending
                            gb, Pb = bl_prev[g]
                            drain_pv(po_prev, gb, Pb, vs_prev)
                            if g == 2:
                                epilogue_a(q_prev, po_prev)
                                py_prev = alloc_py(q_prev)
                        if g == 4 and pending is not None:
                            epilogue_b(pending[0], py_prev)
                            pending = None
                        if len(backlog) >= 4:
                            gb, Pb = backlog.pop(0)
                            drain_pv(po, gb, Pb)
                pending = (q, po, backlog, v_store)
        q_prev, po_prev, bl_prev, vs_prev = pending
        for gb, Pb in bl_prev:
            drain_pv(po_prev, gb, Pb, vs_prev)
        epilogue_a(q_prev, po_prev)
        epilogue_b(q_prev, alloc_py(q_prev))
        pending = None
    nc.finalize()
    return nc


def host_prep(x, interaction_mask, W_qkv, b_qkv, W_out, b_out, n=N):
    """Build per-core input bindings (host-side sharding + layout prep)."""
    import ml_dtypes
    b16 = ml_dtypes.bfloat16
    x = np.asarray(x, np.float32)
    interaction_mask = np.asarray(interaction_mask, np.float32)
    W_qkv = np.asarray(W_qkv, np.float32)
    b_qkv = np.asarray(b_qkv, np.float32)
    W_out = np.asarray(W_out, np.float32)

    maskT = np.ascontiguousarray(
        ((0.1 + 0.9 * interaction_mask) * SCALE).T).astype(b16)
    Wr = W_qkv.reshape(C, 3, H, D)
    br = b_qkv.reshape(3, H, D)
    Wor = W_out.reshape(H, D, C)

    in_maps = []
    for core in range(NCORES):
        b = core // 2
        g = core % 2
        hs = slice(4 * g, 4 * g + 4)
        xT = np.ascontiguousarray(x[b].T).astype(b16)  # [C, n]

        wq = np.concatenate([Wr[:, 0, hs, :].reshape(C, HD),
                             br[0, hs, :].reshape(1, HD)], axis=0)
        wk = np.concatenate([Wr[:, 1, hs, :].reshape(C, HD),
                             br[1, hs, :].reshape(1, HD)], axis=0)
        # v augmented with a ones column per head: weights 0, bias 1
        wv_blocks, bv_blocks = [], []
        for h in range(4 * g, 4 * g + 4):
            wv_blocks.append(np.concatenate(
                [Wr[:, 2, h, :], np.zeros((C, 1), np.float32)], axis=1))
            bv_blocks.append(np.concatenate(
                [br[2, h, :], np.ones((1,), np.float32)]))
        wv = np.concatenate(
            [np.concatenate(wv_blocks, axis=1),
             np.concatenate(bv_blocks)[None, :]], axis=0)  # [C+1, VW]
        wo = np.ascontiguousarray(Wor[hs].reshape(HD, C))

        in_maps.append({
            "xT": xT,
            "maskT": maskT,
            "wq": np.ascontiguousarray(wq).astype(b16),
            "wk": np.ascontiguousarray(wk).astype(b16),
            "wv": np.ascontiguousarray(wv).astype(b16),
            "wo": wo,
        })
    return in_maps


_PROGRAM = {}


def get_program(**kwargs):
    key = tuple(sorted(kwargs.items()))
    if key not in _PROGRAM:
        _PROGRAM[key] = build_program(**kwargs)
    return _PROGRAM[key]


def combine_outputs(results, b_out):
    """results: list of 8 per-core {name: np.ndarray}. Sums head-group
    partials per batch and adds the output bias."""
    b_out = np.asarray(b_out, np.float32)
    out = np.empty((B, N, C), np.float32)
    for b in range(B):
        out[b] = results[2 * b]["y"] + results[2 * b + 1]["y"] + b_out[None, :]
    return out


def kernel(x, interaction_mask, W_qkv, b_qkv, W_out, b_out):
    from concourse.bass_utils import run_bass_kernel_spmd

    in_maps = host_prep(x, interaction_mask, W_qkv, b_qkv, W_out, b_out)
    nc = get_program()
    res = run_bass_kernel_spmd(nc, in_maps, list(range(NCORES)))
    return combine_outputs(res.results, b_out)


# revision 24
# speedup vs baseline: 1.4204x; 1.4204x over previous
"""Trainium2 Bass kernel: BiologicalAttention (mask-modulated multi-head attention).

Full computation:
    qkv = x @ W_qkv + b_qkv                         [B, N, 3, H, D]
    S   = (q @ k^T) * D**-0.5 * (0.1 + 0.9*mask)    [B, H, N, N]
    P   = softmax(S, axis=-1)
    out = (P @ v) reshaped to [B, N, C]
    y   = out @ W_out + b_out

Sharding (8 cores): core c handles batch b = c//2 and a 4-head group
g = c%2 (heads 4g..4g+3).  Each core computes a partial y for its batch
(its heads' contribution to the output projection); the host sums the
two partials per batch and adds b_out.

On-core layout / engine assignment (v2):
  - All attention operands are bf16 (x, q, k, v, mask, P); PSUM
    accumulation stays fp32, output y fp32.  The DVE's fp32 tensor_tensor
    from PSUM runs at 1x (plus a pipe-drain tax), so element work is
    spread: per key-tile the pre-softmax mask multiply runs either on the
    DVE (tensor_mul straight out of PSUM) or on GPSIMD (ACT evicts
    PSUM->SBUF bf16, Q7 multiplies in place).  gpx/16 of tiles go to GPSIMD.
  - Scores are computed TRANSPOSED: T[m, n] = sum_d k[m,d] q[n,d], so the
    softmax denominator (sum over keys m = partitions) comes from a matmul:
    V is stored [m, d] with a ones-column appended, so P@[v|1] yields both
    the attention output (rows 0..31) and the softmax sums (row 32).
  - exp runs on the scalar engine, one op per TE-tile group.
  - P@V for group g is drained three groups late so the PE never waits on
    the DVE/ACT softmax chain; a pass's tail groups (and the whole last
    pass of a rep) drain under the next pass's/rep's leading QK^T work.
    Two heads share each 1-bank PSUM accumulator (tile_position col 0/64),
    so the accumulators double-buffer across passes in 4 banks total.
  - Normalization is deferred past P@V: 1/sums via DVE reciprocal (keeps
    the ACT on a single Exp table set), O_h scaled while evicting PSUM.
  - Host pre-folds scale into the mask: maskT = ((0.1+0.9*mask)*D^-0.5).T
    in bf16, and pre-augments weights with bias rows (x^T gets a ones row).
"""

import numpy as np
from contextlib import ExitStack

import concourse.bass as bass
import concourse.tile as tile
import concourse.mybir as mybir
from concourse import bacc

f32 = mybir.dt.float32
f32r = mybir.dt.float32r
bf16 = mybir.dt.bfloat16
Act = mybir.ActivationFunctionType

# problem shape (hardcoded per contract)
B, N, C, H = 4, 2048, 256, 8
D = 32
SCALE = D ** -0.5
HPC = 4                # heads per core
HD = HPC * D           # 128
VW = HPC * (D + 1)     # 132: per-m-tile v-store width ([v_h | 1] x 4 heads)
NCORES = 8


def build_program(n=N, debug=False, reps=1, gpx=6, act_off=1):
    """Build the SPMD Bass program for one core's shard. Same program runs
    on all 8 cores with different input bindings.

    reps: repeat the whole computation (timing aid).
    gpx: of every 16 key-tiles, gpx are mask-multiplied on GPSIMD (ACT
    evicts PSUM->SBUF, Q7 multiplies in SBUF); the rest on the DVE.
    act_off: move the small epilogue copies (softmax sums, y eviction)
    from the ACT to the DVE.
    """
    NQ = 4                 # n (query) passes
    CH = n // NQ           # 512 at full size
    MT = n // 128          # m-tiles (key tiles)
    TE = 2                 # m-tiles sharing one Exp op
    assert MT % TE == 0

    nc = bacc.Bacc("TRN2", target_bir_lowering=False, debug=debug)

    xT_d = nc.dram_tensor("xT", [C, n], bf16, kind="ExternalInput")
    maskT_d = nc.dram_tensor("maskT", [n, n], bf16, kind="ExternalInput")
    wq_d = nc.dram_tensor("wq", [C + 1, HD], bf16, kind="ExternalInput")
    wk_d = nc.dram_tensor("wk", [C + 1, HD], bf16, kind="ExternalInput")
    wv_d = nc.dram_tensor("wv", [C + 1, VW], bf16, kind="ExternalInput")
    wo_d = nc.dram_tensor("wo", [HD, C], f32, kind="ExternalInput")
    y_d = nc.dram_tensor("y", [n, C], f32, kind="ExternalOutput")

    def is_gp(t):
        return (t * gpx) % MT < gpx

    with tile.TileContext(nc) as tc, ExitStack() as ctx:
        const = ctx.enter_context(tc.tile_pool(name="const", bufs=1))
        maskp = ctx.enter_context(tc.tile_pool(name="maskp", bufs=8))
        tpool = ctx.enter_context(tc.tile_pool(name="tpool", bufs=4))
        ppool = ctx.enter_context(tc.tile_pool(name="ppool", bufs=6))
        ypool = ctx.enter_context(tc.tile_pool(name="ypool", bufs=2))
        spool = ctx.enter_context(tc.tile_pool(name="spool", bufs=2))
        rpool = ctx.enter_context(tc.tile_pool(name="rpool", bufs=2))
        psT = ctx.enter_context(tc.tile_pool(name="psT", bufs=2, space="PSUM"))
        # two 2-bank pools; po(q) lives in pool q%2, and pass q's projection
        # output py(q) reuses the same slots right after the normalize reads,
        # so consecutive passes' PV accumulators never share banks.
        psA = ctx.enter_context(tc.tile_pool(name="psA", bufs=2, space="PSUM"))
        psB = ctx.enter_context(tc.tile_pool(name="psB", bufs=2, space="PSUM"))
        psO2 = (psA, psB)

        # ---------------- constants / inputs ----------------
        xb0 = const.tile([128, n], bf16, tag="xb0")
        xb1 = const.tile([128, n], bf16, tag="xb1")
        nc.sync.dma_start(xb0[:], xT_d[0:128, :])
        nc.sync.dma_start(xb1[:], xT_d[128:256, :])
        ones_b = const.tile([1, n], bf16, tag="ones_b")
        nc.vector.memset(ones_b[:], 1.0)
        zc = const.tile([1, 128], bf16, tag="zc")
        nc.vector.memset(zc[:], 0.0)
        zrow = const.tile([1, CH], bf16, tag="zrow")
        nc.vector.memset(zrow[:], 0.0)

        wq_sb = const.tile([128, 2 * HD], bf16, tag="wq_sb")
        wk_sb = const.tile([128, 2 * HD], bf16, tag="wk_sb")
        wv_sb = const.tile([128, 2 * VW], bf16, tag="wv_sb")
        wqb = const.tile([1, HD], bf16, tag="wqb")
        wkb = const.tile([1, HD], bf16, tag="wkb")
        wvb = const.tile([1, VW], bf16, tag="wvb")
        wo_f = const.tile([128, C], f32, tag="wo_f")
        wo_sb = const.tile([128, C], f32r, tag="wo_sb")
        for sb, d_, w in ((wq_sb, wq_d, HD), (wk_sb, wk_d, HD), (wv_sb, wv_d, VW)):
            nc.sync.dma_start(sb[:, 0:w], d_[0:128, :])
            nc.sync.dma_start(sb[:, w:2 * w], d_[128:256, :])
        nc.sync.dma_start(wqb[:], wq_d[256:257, :])
        nc.sync.dma_start(wkb[:], wk_d[256:257, :])
        nc.sync.dma_start(wvb[:], wv_d[256:257, :])
        nc.sync.dma_start(wo_f[:], wo_d[:])
        nc.scalar.copy(wo_sb[:], wo_f[:])

        qT_sb = const.tile([128, n], bf16, tag="qT_sb")
        kT_sb = const.tile([128, n], bf16, tag="kT_sb")
        # v_store is double-buffered by rep parity: the tail P@V of rep i
        # drains under rep i+1's phase 1, which writes the other buffer.
        v_stores = [const.tile([128, MT * VW], bf16, name=f"v_store{i}",
                               tag=f"v_store{i}") for i in range(2)]
        O_allT = const.tile([128, n], f32r, tag="O_allT")

        pending = None
        po = None
        for _rep in range(reps):
            v_store = v_stores[_rep % 2]
            # ------------- phase 1: QKV projections (bf16) -------------
            # qT/kT: [32h+d, n] = W.T @ x.T (+ bias via ones-row rank-1 term)
            for chunk in range(NQ):
                cs = bass.ts(chunk, CH)
                for dst, w_sb, w_b in ((qT_sb, wq_sb, wqb), (kT_sb, wk_sb, wkb)):
                    pq = psT.tile([128, CH], f32, name="pq", tag="psT")
                    nc.tensor.matmul(pq[:], lhsT=w_sb[:, 0:HD], rhs=xb0[:, cs],
                                     start=True, stop=False)
                    nc.tensor.matmul(pq[:], lhsT=w_sb[:, HD:2 * HD],
                                     rhs=xb1[:, cs], start=False, stop=False)
                    nc.tensor.matmul(pq[:], lhsT=w_b[0:1, :],
                                     rhs=ones_b[0:1, cs],
                                     start=False, stop=True)
                    nc.vector.tensor_copy(dst[:, cs], pq[:])
            # v: [m, (v_h | 1) x 4] per m-tile; ones col comes from the bias row
            for t in range(MT):
                ms = bass.ts(t, 128)
                pv = psT.tile([128, VW], f32, name="pv", tag="psT")
                nc.tensor.matmul(pv[:], lhsT=xb0[:, ms], rhs=wv_sb[:, 0:VW],
                                 start=True, stop=False)
                nc.tensor.matmul(pv[:], lhsT=xb1[:, ms],
                                 rhs=wv_sb[:, VW:2 * VW],
                                 start=False, stop=False)
                nc.tensor.matmul(pv[:], lhsT=ones_b[0:1, ms], rhs=wvb[0:1, :],
                                 start=False, stop=True)
                nc.vector.tensor_copy(v_store[:, t * VW:(t + 1) * VW],
                                      pv[:])

            # ------------- phase 2: attention, one pass per n-chunk ----------
            # po is 2 PSUM banks (double-buffered across passes): heads h
            # pack two-per-bank at partition offsets 0/64 via tile_position,
            # so the normalize epilogue of pass q overlaps pass q+1 fully.
            def epilogue_a(q, po):
                qs = bass.ts(q, CH)
                # sums (psum row 32+64j of each head) -> partition 32h
                sraw = spool.tile([128, CH], f32, name="sraw", tag="sraw")
                sums_copy = (nc.vector.tensor_copy if act_off
                             else nc.scalar.copy)
                for h in range(4):
                    sums_copy(sraw[32 * h:32 * h + 1, :],
                              po[h // 2][64 * (h % 2) + 32:
                                         64 * (h % 2) + 33, :])
                # broadcast each head's sums row across its 32 partitions (DMA)
                r_raw = rpool.tile([128, CH], f32, name="r_raw", tag="r_raw")
                src = sraw[:]
                bc = bass.AP(tensor=src.tensor, offset=src.offset,
                             ap=[[32 * src.ap[0][0], 4], [0, 32], src.ap[-1]])
                nc.sync.dma_start(r_raw[:], bc)
                r_all = rpool.tile([128, CH], f32, name="r_all", tag="r_all")
                nc.vector.reciprocal(r_all[:], r_raw[:])
                # evict + normalize O^T chunks (rounds to f32r for projection)
                for h in range(4):
                    nc.vector.tensor_mul(O_allT[32 * h:32 * h + 32, qs],
                                         po[h // 2][64 * (h % 2):
                                                    64 * (h % 2) + 32, :],
                                         r_all[32 * h:32 * h + 32, :])

            def alloc_py(q):
                # projection PSUM for pass q: the same pool slots po(q) used
                # (free right after epilogue_a's normalize reads)
                pool = psO2[q % 2]
                return [pool.tile([128, CH], f32, name="py",
                                  tag=f"ps{'AB'[q % 2]}") for _ in range(2)]

            def epilogue_b(q, py):
                # output projection for this n-chunk
                for j in range(CH // 128):
                    ncol = q * CH + j * 128
                    nc.tensor.matmul(py[j // 2][:, (j % 2) * C:
                                                (j % 2 + 1) * C],
                                     lhsT=O_allT[:, ncol:ncol + 128],
                                     rhs=wo_sb[:], start=True, stop=True)
                y_sb = ypool.tile([128, (CH // 128) * C], f32, name="y_sb",
                                  tag="y_sb")
                y_copy = (nc.vector.tensor_copy if act_off
                          else nc.scalar.copy)
                y_copy(y_sb[:, 0:2 * C], py[0][:])
                y_copy(y_sb[:, 2 * C:4 * C], py[1][:])
                for j in range(CH // 128):
                    nc.sync.dma_start(
                        y_d[q * CH + j * 128:q * CH + (j + 1) * 128, :],
                        y_sb[:, j * C:(j + 1) * C])

            def drain_pv(po, g, P_t, vst=None):
                # P@V for TE-group g (tiles TE*g .. TE*g+TE-1).  Two heads
                # share a PSUM bank at partition offsets 0/64; the bank was
                # zero-initialized at alloc so every matmul accumulates.
                if vst is None:
                    vst = v_store
                for tj in range(TE):
                    tb = TE * g + tj
                    for h in range(4):
                        vs = vst[:, tb * VW + 33 * h:
                                 tb * VW + 33 * h + 33]
                        off = 64 * (h % 2)
                        nc.tensor.matmul(
                            po[h // 2][off:off + 33, :],
                            lhsT=vs,
                            rhs=P_t[:, (tj * 4 + h) * CH:
                                    (tj * 4 + h + 1) * CH],
                            start=False,
                            stop=(tb == MT - 1),
                            tile_position=(0, off),
                            skip_group_check=True)

            for q in range(NQ):
                backlog = []
                for t in range(MT):
                    mask_t = maskp.tile([128, CH], bf16, tag="mask_t")
                    nc.sync.dma_start(
                        mask_t[:],
                        maskT_d[t * 128:(t + 1) * 128, q * CH:(q + 1) * CH])
                    ti = t % TE
                    if ti == 0:
                        T_tile = tpool.tile([128, TE * 4 * CH], bf16,
                                            name="T_tile", tag="T_tile")
                        P_tile = ppool.tile([128, TE * 4 * CH], bf16,
                                            name="P_tile", tag="P_tile")
                    for pair in range(2):
                        pt = psT.tile([128, 2 * CH], f32, name="pt", tag="psT")
                        for hh in range(2):
                            h = 2 * pair + hh
                            nc.tensor.matmul(
                                pt[:, hh * CH:(hh + 1) * CH],
                                lhsT=kT_sb[32 * h:32 * h + 32,
                                           t * 128:(t + 1) * 128],
                                rhs=qT_sb[32 * h:32 * h + 32,
                                          bass.ts(q, CH)],
                                start=True, stop=True,
                                tile_position=(32 * h, 0))
                        # fused PSUM-evict + mask multiply (mask repeated 2x)
                        mrep = bass.AP(tensor=mask_t[:].tensor,
                                       offset=mask_t[:].offset,
                                       ap=[mask_t[:].ap[0], [0, 2],
                                           mask_t[:].ap[-1]])
                        dst = T_tile[:, (ti * 4 + pair * 2) * CH:
                                     (ti * 4 + pair * 2 + 2) * CH]
                        if is_gp(t):
                            # relieve the DVE: ACT evicts PSUM, GPSIMD
                            # does the elementwise multiply in SBUF
                            nc.scalar.copy(dst, pt[:])
                            nc.gpsimd.tensor_mul(dst, dst, mrep)
                        else:
                            nc.vector.tensor_mul(dst, pt[:], mrep)
                    if ti == TE - 1:
                        g = t // TE
                        nc.scalar.activation(P_tile[:], T_tile[:], Act.Exp)
                        backlog.append((g, P_tile))
                        if g == 0:
                            # previous pass's P@V tail (three groups) and
                            # its epilogue run here, hidden behind this
                            # pass's first QK^T groups
                            py_prev = None
                            po = [psO2[q % 2].tile(
                                [128, CH], f32, name="po",
                                tag=f"ps{'AB'[q % 2]}") for _ in range(2)]
                            for pb in po:
                                nc.tensor.matmul(pb[:], lhsT=zc[0:1, :],
                                                 rhs=zrow[0:1, :],
                                                 start=True, stop=True)
                        if g <= 2 and pending is not None:
                            q_prev, po_prev, bl_prev, vs_prev = pending
                            gb, Pb = bl_prev[g]
                            drain_pv(po_prev, gb, Pb, vs_prev)
                            if g == 2:
                                epilogue_a(q_prev, po_prev)
                                py_prev = alloc_py(q_prev)
                        if g == 4 and pending is not None:
                            epilogue_b(pending[0], py_prev)
                            pending = None
                        if len(backlog) >= 4:
                            gb, Pb = backlog.pop(0)
                            drain_pv(po, gb, Pb)
                pending = (q, po, backlog, v_store)
        q_prev, po_prev, bl_prev, vs_prev = pending
        for gb, Pb in bl_prev:
            drain_pv(po_prev, gb, Pb, vs_prev)
        epilogue_a(q_prev, po_prev)
        epilogue_b(q_prev, alloc_py(q_prev))
        pending = None
    nc.finalize()
    return nc


def host_prep(x, interaction_mask, W_qkv, b_qkv, W_out, b_out, n=N):
    """Build per-core input bindings (host-side sharding + layout prep)."""
    import ml_dtypes
    b16 = ml_dtypes.bfloat16
    x = np.asarray(x, np.float32)
    interaction_mask = np.asarray(interaction_mask, np.float32)
    W_qkv = np.asarray(W_qkv, np.float32)
    b_qkv = np.asarray(b_qkv, np.float32)
    W_out = np.asarray(W_out, np.float32)

    maskT = np.ascontiguousarray(
        ((0.1 + 0.9 * interaction_mask) * SCALE).T).astype(b16)
    Wr = W_qkv.reshape(C, 3, H, D)
    br = b_qkv.reshape(3, H, D)
    Wor = W_out.reshape(H, D, C)

    in_maps = []
    for core in range(NCORES):
        b = core // 2
        g = core % 2
        hs = slice(4 * g, 4 * g + 4)
        xT = np.ascontiguousarray(x[b].T).astype(b16)  # [C, n]

        wq = np.concatenate([Wr[:, 0, hs, :].reshape(C, HD),
                             br[0, hs, :].reshape(1, HD)], axis=0)
        wk = np.concatenate([Wr[:, 1, hs, :].reshape(C, HD),
                             br[1, hs, :].reshape(1, HD)], axis=0)
        # v augmented with a ones column per head: weights 0, bias 1
        wv_blocks, bv_blocks = [], []
        for h in range(4 * g, 4 * g + 4):
            wv_blocks.append(np.concatenate(
                [Wr[:, 2, h, :], np.zeros((C, 1), np.float32)], axis=1))
            bv_blocks.append(np.concatenate(
                [br[2, h, :], np.ones((1,), np.float32)]))
        wv = np.concatenate(
            [np.concatenate(wv_blocks, axis=1),
             np.concatenate(bv_blocks)[None, :]], axis=0)  # [C+1, VW]
        wo = np.ascontiguousarray(Wor[hs].reshape(HD, C))

        in_maps.append({
            "xT": xT,
            "maskT": maskT,
            "wq": np.ascontiguousarray(wq).astype(b16),
            "wk": np.ascontiguousarray(wk).astype(b16),
            "wv": np.ascontiguousarray(wv).astype(b16),
            "wo": wo,
        })
    return in_maps


_PROGRAM = {}


def get_program(**kwargs):
    key = tuple(sorted(kwargs.items()))
    if key not in _PROGRAM:
        _PROGRAM[key] = build_program(**kwargs)
    return _PROGRAM[key]


def combine_outputs(results, b_out):
    """results: list of 8 per-core {name: np.ndarray}. Sums head-group
    partials per batch and adds the output bias."""
    b_out = np.asarray(b_out, np.float32)
    out = np.empty((B, N, C), np.float32)
    for b in range(B):
        out[b] = results[2 * b]["y"] + results[2 * b + 1]["y"] + b_out[None, :]
    return out


def kernel(x, interaction_mask, W_qkv, b_qkv, W_out, b_out):
    from concourse.bass_utils import run_bass_kernel_spmd

    in_maps = host_prep(x, interaction_mask, W_qkv, b_qkv, W_out, b_out)
    nc = get_program()
    res = run_bass_kernel_spmd(nc, in_maps, list(range(NCORES)))
    return combine_outputs(res.results, b_out)
